# revision 1
# baseline (speedup 1.0000x reference)
"""DSMIL forward pass on 8 Trainium2 NeuronCores (Bass/Tile).

Sharding: data-parallel over bags with each bag split across a core pair
(core 2b gets instances [0:4096) of bag b, core 2b+1 gets [4096:8192)).
The cross-half argmax winner and the softmax partial sums are exchanged
through two tiny pair-local collectives (AllGather + AllReduce), so the
whole computation is a single NEFF launch.

Per-core pipeline:
  phase 1 (streamed over chunks of CHUNK instances):
    x chunk -> PE-transpose -> xT ; hT = relu(W_enc^T @ xT) (fp32r matmuls);
    h_nat via PE-transpose of hT; classes^T (+ natural-layout copy),
    Z^T -> Q^T = tanh(...)
  phase 2:
    per-class max over classes^T (DVE max), onehot == max, critical
    instance features m = onehot^T @ h_nat, candidate q_max = q_fn(m);
    AllGather (maxval, q_max) within the pair; winner select;
    scores = q_win^T @ Q^T -> exp (with accumulated denominator);
    partial B = e^T @ h_nat; AllReduce partials; B = num / den.
"""
import numpy as np
from contextlib import ExitStack

import concourse.bacc as bacc
import concourse.tile as tile
import concourse.mybir as mybir

F32 = mybir.dt.float32
F32R = mybir.dt.float32r
AF = mybir.ActivationFunctionType
ALU = mybir.AluOpType

N_CORES = 8
B_BAGS = 4
N_FULL = 8192
N_LOC = N_FULL // 2

_cache = {}


def _build_kernel(n_cores=N_CORES, N_loc=N_LOC, I=1024, D=512, QD=128,
                  C=2, CHUNK=512):
    NB = N_loc // 128          # n-blocks
    NCH = N_loc // CHUNK       # chunks
    BPC = CHUNK // 128         # n-blocks per chunk
    IB = I // 128              # i-blocks
    DB = D // 128              # d-blocks
    SCH = 512                  # score/exp chunk
    NSC = N_loc // SCH
    assert QD == 128 and C == 2
    inv_sqrt_q = 1.0 / float(np.sqrt(QD))

    nc = bacc.Bacc("TRN2", target_bir_lowering=False, debug=False,
                   num_devices=n_cores)

    xh = nc.dram_tensor("xh", [N_loc, I], F32, kind="ExternalInput")
    w_enc = nc.dram_tensor("w_enc", [I, D], F32, kind="ExternalInput")
    b_enc = nc.dram_tensor("b_enc", [DB, 128, 1], F32, kind="ExternalInput")
    w_i = nc.dram_tensor("w_i", [D, C], F32, kind="ExternalInput")
    b_i = nc.dram_tensor("b_i", [C, 1], F32, kind="ExternalInput")
    w_q1 = nc.dram_tensor("w_q1", [D, QD], F32, kind="ExternalInput")
    b_q1 = nc.dram_tensor("b_q1", [QD, 1], F32, kind="ExternalInput")
    w_q2 = nc.dram_tensor("w_q2", [QD, QD], F32, kind="ExternalInput")
    b_q2 = nc.dram_tensor("b_q2", [QD, 1], F32, kind="ExternalInput")
    ident_d = nc.dram_tensor("ident", [128, 128], F32, kind="ExternalInput")
    out_d = nc.dram_tensor("out", [C, D], F32, kind="ExternalOutput")

    groups = [[i, i + 1] for i in range(0, n_cores, 2)]

    with tile.TileContext(nc) as tc, ExitStack() as ctx:
        persist = ctx.enter_context(tc.tile_pool(name="persist", bufs=1))
        dram = ctx.enter_context(tc.tile_pool(name="dram", bufs=1,
                                              space="DRAM"))

        # warm up the collective channels while phase 1 runs
        warm_in = dram.tile([1, 2], F32)
        warm_out = dram.tile([2, 2], F32)
        nc.sync.dma_start(warm_in[:], ident_d[0:1, 0:2])
        nc.gpsimd.collective_compute(
            "AllGather", ALU.bypass, replica_groups=groups,
            ins=[warm_in[:].opt()], outs=[warm_out[:].opt()])

        ident_rt = persist.tile([128, 128], F32R)
        nc.gpsimd.dma_start(ident_rt[:], ident_d[:])
        ident_r = ident_rt[:]
        ident_ft = persist.tile([128, 128], F32)
        nc.sync.dma_start(ident_ft[:], ident_d[:])
        ident_f = ident_ft[:]
        ident2t = persist.tile([2, 2], F32)
        nc.sync.dma_start(ident2t[:], ident_d[0:2, 0:2])
        ident2 = ident2t[:]
        ones_t = persist.tile([1, 128], F32)
        nc.gpsimd.memset(ones_t[:], 1.0)

        w_enc_sb = []
        for ib in range(IB):
            t = persist.tile([128, D], F32R, name=f"wenc{ib}")
            nc.gpsimd.dma_start(t[:], w_enc[ib * 128:(ib + 1) * 128, :])
            w_enc_sb.append(t)
        w_q1_sb = []
        for db in range(DB):
            t = persist.tile([128, QD], F32R, name=f"wq1{db}")
            nc.gpsimd.dma_start(t[:], w_q1[db * 128:(db + 1) * 128, :])
            w_q1_sb.append(t)
        w_q2_sb = persist.tile([128, QD], F32R)
        nc.gpsimd.dma_start(w_q2_sb[:], w_q2[:])
        w_i_sb = []
        for db in range(DB):
            t = persist.tile([128, C], F32R, name=f"wi{db}")
            nc.gpsimd.dma_start(t[:], w_i[db * 128:(db + 1) * 128, :])
            w_i_sb.append(t)
        b_enc_sb = []
        for db in range(DB):
            t = persist.tile([128, 1], F32, name=f"benc{db}")
            nc.sync.dma_start(t[:], b_enc[db])
            b_enc_sb.append(t)
        b_i_sb = persist.tile([C, 1], F32)
        nc.sync.dma_start(b_i_sb[:], b_i[:])
        b_q1_sb = persist.tile([QD, 1], F32)
        nc.sync.dma_start(b_q1_sb[:], b_q1[:])
        b_q2_sb = persist.tile([QD, 1], F32)
        nc.sync.dma_start(b_q2_sb[:], b_q2[:])

        h_nat = persist.tile([128, NB * D], F32R)
        qt_sb = persist.tile([128, N_loc], F32R)
        cls_t = persist.tile([C, N_loc], F32)
        cls_nat = persist.tile([128, NB * C], F32)
        oh_sb = persist.tile([128, NB * C], F32R)
        e_nat = persist.tile([128, NB * C], F32R)

        # ================= phase 1: encoder streaming =================
        with (
            tc.tile_pool(name="xload", bufs=2) as xload,
            tc.tile_pool(name="xtp", bufs=1) as xtp,
            tc.tile_pool(name="htp", bufs=2) as htp,
            tc.tile_pool(name="ztp", bufs=2) as ztp,
            tc.tile_pool(name="pt", bufs=2, space="PSUM") as pt_pool,
            tc.tile_pool(name="ph", bufs=1, space="PSUM") as ph_pool,
            tc.tile_pool(name="paux", bufs=2, space="PSUM") as paux,
        ):
            for cb in range(NCH):
                n0 = cb * CHUNK
                x_nat = xload.tile([128, BPC, I], F32, tag="xn", name="xn")
                src = xh[n0:n0 + CHUNK, :].rearrange("(b p) i -> p b i",
                                                     p=128)
                nc.sync.dma_start(x_nat[:], src)

                xt = [xtp.tile([128, CHUNK], F32R, tag=f"xt{ib}",
                               name=f"xt{ib}") for ib in range(IB)]
                for ib in range(IB):
                    for b in range(BPC):
                        ptile = pt_pool.tile([128, 128], F32, tag="pt",
                                             name="pt")
                        nc.tensor.transpose(
                            ptile[:], x_nat[:, b, ib * 128:(ib + 1) * 128],
                            ident_f)
                        if (ib + b) % 2 == 0:
                            nc.vector.tensor_copy(
                                xt[ib][:, b * 128:(b + 1) * 128], ptile[:])
                        else:
                            nc.scalar.copy(
                                xt[ib][:, b * 128:(b + 1) * 128], ptile[:])

                ht = [htp.tile([128, CHUNK], F32R, tag=f"ht{db}",
                               name=f"ht{db}") for db in range(DB)]
                for db in range(DB):
                    ph = ph_pool.tile([128, CHUNK], F32, tag=f"ph{db}",
                                      name=f"ph{db}")
                    for ib in range(IB):
                        nc.tensor.matmul(
                            ph[:], w_enc_sb[ib][:, db * 128:(db + 1) * 128],
                            xt[ib][:], start=(ib == 0), stop=(ib == IB - 1))
                    nc.scalar.activation(ht[db][:], ph[:], AF.Relu,
                                         bias=b_enc_sb[db][:])
                    for b in range(BPC):
                        nb = cb * BPC + b
                        ptile = pt_pool.tile([128, 128], F32R, tag="pt",
                                             name="pt")
                        nc.tensor.transpose(
                            ptile[:], ht[db][:, b * 128:(b + 1) * 128],
                            ident_r)
                        nc.vector.tensor_copy(
                            h_nat[:, nb * D + db * 128:
                                  nb * D + (db + 1) * 128],
                            ptile[:])

                pc = paux.tile([C, CHUNK], F32, tag="aux", name="pc")
                for db in range(DB):
                    nc.tensor.matmul(pc[:], w_i_sb[db][:], ht[db][:],
                                     start=(db == 0), stop=(db == DB - 1))
                nc.scalar.activation(cls_t[:, n0:n0 + CHUNK], pc[:],
                                     AF.Identity, bias=b_i_sb[:])
                # natural-layout classes for the onehot comparison later
                for b in range(BPC):
                    nb = cb * BPC + b
                    ptn = paux.tile([128, C], F32, tag="aux", name="ptn")
                    nc.tensor.transpose(
                        ptn[:], cls_t[:, nb * 128:(nb + 1) * 128], ident2)
                    nc.vector.tensor_copy(
                        cls_nat[:, nb * C:(nb + 1) * C], ptn[:])

                pz = paux.tile([128, CHUNK], F32, tag="aux", name="pz")
                for db in range(DB):
                    nc.tensor.matmul(pz[:], w_q1_sb[db][:], ht[db][:],
                                     start=(db == 0), stop=(db == DB - 1))
                zt = ztp.tile([128, CHUNK], F32R, tag="zt", name="zt")
                nc.scalar.activation(zt[:], pz[:], AF.Relu, bias=b_q1_sb[:])
                pq = paux.tile([128, CHUNK], F32, tag="aux", name="pq")
                nc.tensor.matmul(pq[:], w_q2_sb[:], zt[:],
                                 start=True, stop=True)
                nc.scalar.activation(qt_sb[:, n0:n0 + CHUNK], pq[:],
                                     AF.Tanh, bias=b_q2_sb[:])

        # ================= phase 2 =================
        with (
            tc.tile_pool(name="p2sb", bufs=1) as p2,
            tc.tile_pool(name="psmall", bufs=3, space="PSUM") as psmall,
            tc.tile_pool(name="psc", bufs=2, space="PSUM") as psc,
            tc.tile_pool(name="pbig", bufs=1, space="PSUM") as pbig,
        ):
            cmax8 = p2.tile([C, 8], F32)
            nc.vector.max(cmax8[:], cls_t[:])
            pmv = psmall.tile([8, C], F32, tag="small", name="pmv")
            nc.tensor.transpose(pmv[:], cmax8[:], ident2)
            mval_f = p2.tile([1, C], F32)
            nc.vector.tensor_copy(mval_f[:], pmv[0:1, :])
            pmb = psmall.tile([128, C], F32, tag="small", name="pmb")
            nc.tensor.matmul(pmb[:], ones_t[:], mval_f[:],
                             start=True, stop=True)
            mb = p2.tile([128, C], F32)
            nc.vector.tensor_copy(mb[:], pmb[:])

            for nb in range(NB):
                nc.vector.tensor_tensor(oh_sb[:, nb * C:(nb + 1) * C],
                                        cls_nat[:, nb * C:(nb + 1) * C],
                                        mb[:], ALU.is_equal)

            pmf = pbig.tile([C, D], F32, tag="mf", name="pmf")
            for nb in range(NB):
                nc.tensor.matmul(pmf[:], oh_sb[:, nb * C:(nb + 1) * C],
                                 h_nat[:, nb * D:(nb + 1) * D],
                                 start=(nb == 0), stop=(nb == NB - 1))
            mf_nat = p2.tile([C, D], F32)
            nc.vector.tensor_copy(mf_nat[:], pmf[:])
            mfT = p2.tile([128, DB * C], F32R)
            for db in range(DB):
                ptm = psmall.tile([128, C], F32, tag="small", name="ptm")
                nc.tensor.transpose(ptm[:],
                                    mf_nat[:, db * 128:(db + 1) * 128],
                                    ident2)
                nc.vector.tensor_copy(mfT[:, db * C:(db + 1) * C], ptm[:])
            pzm = psmall.tile([128, C], F32, tag="small", name="pzm")
            for db in range(DB):
                nc.tensor.matmul(pzm[:], w_q1_sb[db][:],
                                 mfT[:, db * C:(db + 1) * C],
                                 start=(db == 0), stop=(db == DB - 1))
            zm = p2.tile([128, C], F32R)
            nc.scalar.activation(zm[:], pzm[:], AF.Relu, bias=b_q1_sb[:])
            pqc = psmall.tile([128, C], F32, tag="small", name="pqc")
            nc.tensor.matmul(pqc[:], w_q2_sb[:], zm[:], start=True, stop=True)
            qcand = p2.tile([128, C], F32)
            nc.scalar.activation(qcand[:], pqc[:], AF.Tanh, bias=b_q2_sb[:])

            pay1 = dram.tile([1 + 128, C], F32)
            nc.sync.dma_start(pay1[0:1, :], mval_f[:])
            nc.sync.dma_start(pay1[1:129, :], qcand[:])
            gath1 = dram.tile([2 * 129, C], F32)
            nc.gpsimd.collective_compute(
                "AllGather", ALU.bypass, replica_groups=groups,
                ins=[pay1[:].opt()], outs=[gath1[:].opt()])

            mv_f = p2.tile([1, 2 * C], F32)
            nc.sync.dma_start(mv_f[:, 0:C], gath1[0:1, :])
            nc.sync.dma_start(mv_f[:, C:2 * C], gath1[129:130, :])
            qA = p2.tile([128, C], F32)
            nc.sync.dma_start(qA[:], gath1[1:129, :])
            qB = p2.tile([128, C], F32)
            nc.sync.dma_start(qB[:], gath1[130:258, :])

            pmb2 = psmall.tile([128, 2 * C], F32, tag="small", name="pmb2")
            nc.tensor.matmul(pmb2[:], ones_t[:], mv_f[:],
                             start=True, stop=True)
            mvb = p2.tile([128, 2 * C], F32)
            nc.vector.tensor_copy(mvb[:], pmb2[:])
            wA = p2.tile([128, C], F32)
            nc.vector.tensor_tensor(wA[:], mvb[:, 0:C], mvb[:, C:2 * C],
                                    ALU.is_ge)
            tdiff = p2.tile([128, C], F32)
            nc.vector.tensor_tensor(tdiff[:], qA[:], qB[:], ALU.subtract)
            tsel = p2.tile([128, C], F32)
            nc.vector.tensor_tensor(tsel[:], tdiff[:], wA[:], ALU.mult)
            q_win = p2.tile([128, C], F32R)
            nc.vector.tensor_tensor(q_win[:], tsel[:], qB[:], ALU.add)

            den8 = p2.tile([C, max(NSC, 2)], F32)
            for sc in range(NSC):
                ps_s = psc.tile([C, SCH], F32, tag="sc", name="ps_s")
                nc.tensor.matmul(ps_s[:], q_win[:],
                                 qt_sb[:, sc * SCH:(sc + 1) * SCH],
                                 start=True, stop=True)
                eT = p2.tile([C, SCH], F32, tag="eT", bufs=2, name="eT")
                nc.scalar.activation(eT[:], ps_s[:], AF.Exp,
                                     scale=inv_sqrt_q,
                                     accum_out=den8[:, sc:sc + 1])
                for k in range(SCH // 128):
                    nb = sc * (SCH // 128) + k
                    pte = psmall.tile([128, C], F32, tag="small", name="pte")
                    nc.tensor.transpose(pte[:], eT[:, k * 128:(k + 1) * 128],
                                        ident2)
                    nc.vector.tensor_copy(e_nat[:, nb * C:(nb + 1) * C],
                                          pte[:])

            den = p2.tile([C, 1], F32)
            nc.vector.reduce_sum(den[:], den8[:, 0:NSC],
                                 axis=mybir.AxisListType.X)

            pnum = pbig.tile([C, D], F32, tag="mf", name="pnum")
            for nb in range(NB):
                nc.tensor.matmul(pnum[:], e_nat[:, nb * C:(nb + 1) * C],
                                 h_nat[:, nb * D:(nb + 1) * D],
                                 start=(nb == 0), stop=(nb == NB - 1))
            num = p2.tile([C, D], F32)
            nc.vector.tensor_copy(num[:], pnum[:])

            pay2 = dram.tile([C, D + 1], F32)
            nc.sync.dma_start(pay2[:, 0:D], num[:])
            nc.sync.dma_start(pay2[:, D:D + 1], den[:])
            red2 = dram.tile([C, D + 1], F32)
            nc.gpsimd.collective_compute(
                "AllReduce", ALU.add, replica_groups=groups,
                ins=[pay2[:].opt()], outs=[red2[:].opt()])
            num_s = p2.tile([C, D], F32)
            nc.sync.dma_start(num_s[:], red2[:, 0:D])
            den_s = p2.tile([C, 1], F32)
            nc.sync.dma_start(den_s[:], red2[:, D:D + 1])

            recip = p2.tile([C, 1], F32)
            nc.vector.reciprocal(recip[:], den_s[:])
            out_sb = p2.tile([C, D], F32)
            nc.vector.tensor_scalar_mul(out_sb[:], num_s[:], recip[:])
            nc.sync.dma_start(out_d[:], out_sb[:])

    nc.compile()
    return nc


def _make_in_maps(inputs, n_cores=N_CORES, N_loc=N_LOC):
    x = np.ascontiguousarray(np.asarray(inputs["x"], dtype=np.float32))
    B = x.shape[0]
    D = int(np.asarray(inputs["W_enc"]).shape[1])
    DB = D // 128
    shared = {
        "w_enc": np.ascontiguousarray(np.asarray(inputs["W_enc"],
                                                 np.float32)),
        "b_enc": np.ascontiguousarray(
            np.asarray(inputs["b_enc"], np.float32).reshape(DB, 128, 1)),
        "w_i": np.ascontiguousarray(np.asarray(inputs["W_i"], np.float32)),
        "ident": np.eye(128, dtype=np.float32),
        "b_i": np.ascontiguousarray(
            np.asarray(inputs["b_i"], np.float32).reshape(-1, 1)),
        "w_q1": np.ascontiguousarray(np.asarray(inputs["W_q1"], np.float32)),
        "b_q1": np.ascontiguousarray(
            np.asarray(inputs["b_q1"], np.float32).reshape(-1, 1)),
        "w_q2": np.ascontiguousarray(np.asarray(inputs["W_q2"], np.float32)),
        "b_q2": np.ascontiguousarray(
            np.asarray(inputs["b_q2"], np.float32).reshape(-1, 1)),
    }
    in_maps = []
    for core in range(n_cores):
        bag = core // 2
        half = core % 2
        xhs = np.ascontiguousarray(
            x[bag % B, half * N_loc:(half + 1) * N_loc, :])
        in_maps.append({"xh": xhs, **shared})
    return in_maps


def kernel(**inputs) -> np.ndarray:
    from concourse.bass_utils import run_bass_kernel_spmd

    if "nc" not in _cache:
        _cache["nc"] = _build_kernel()
    nc = _cache["nc"]
    in_maps = _make_in_maps(inputs)
    res = run_bass_kernel_spmd(nc, in_maps, core_ids=list(range(N_CORES)))
    out = np.stack([res.results[2 * b]["out"] for b in range(B_BAGS)])
    return out.astype(np.float32)



# revision 4
# speedup vs baseline: 1.7955x; 1.7955x over previous
"""DSMIL forward pass on 8 Trainium2 NeuronCores (Bass/Tile), bf16 compute.

Sharding: data-parallel over bags, each bag split across a core pair
(core 2b: instances [0:4096) of bag b, core 2b+1: [4096:8192)). Two tiny
pair-local collectives (argmax exchange + softmax partial reduction) keep
it a single NEFF launch.

Key implementation choices (vs the fp32r baseline):
  - x is transposed and cast to bf16 on the HOST: the kernel streams
    xT i-block tiles straight into matmuls (no on-chip x transposes,
    half the HBM traffic).
  - all matmuls run in bf16 (fp32 PSUM accumulation); the class-score
    path stays fp32 from PSUM onward so the per-class argmax is stable.
  - classes are computed in NATURAL [n, c] layout directly
    (lhsT = h^T block, rhs = W_i block), so the per-class max reduction
    runs on all 128 DVE lanes instead of 2.
  - q_fn(critical instance), attention scores and exp() also produce
    natural-layout tiles, avoiding partition-starved ops.
  - the Q = q_fn(h) pass is deferred until after the AllGather is
    issued, so the collective's sync latency is hidden under real work;
    a small stream of dummy PE transposes afterwards keeps the HAM
    clock-gate warm through the remaining wait.
"""
import numpy as np
import ml_dtypes
from contextlib import ExitStack

import concourse.bacc as bacc
import concourse.tile as tile
import concourse.mybir as mybir

F32 = mybir.dt.float32
BF16 = mybir.dt.bfloat16
AF = mybir.ActivationFunctionType
ALU = mybir.AluOpType
bfdt = ml_dtypes.bfloat16

N_CORES = 8
B_BAGS = 4
N_FULL = 8192
N_LOC = N_FULL // 2

_cache = {}


def _build_kernel(n_cores=N_CORES, N_loc=N_LOC, I=1024, D=512, QD=128,
                  C=2, CHUNK=512, N_WARM=56):
    NB = N_loc // 128          # n-blocks (32)
    NCH = N_loc // CHUNK       # chunks (8)
    BPC = CHUNK // 128         # n-blocks per chunk (4)
    IB = I // 128              # i-blocks (8)
    DB = D // 128              # d-blocks (4)
    assert QD == 128 and C == 2
    inv_sqrt_q = 1.0 / float(np.sqrt(QD))

    nc = bacc.Bacc("TRN2", target_bir_lowering=False, debug=False,
                   num_devices=n_cores)

    xt_d = nc.dram_tensor("xt", [I, N_loc], BF16, kind="ExternalInput")
    w_enc = nc.dram_tensor("w_enc", [I, D], BF16, kind="ExternalInput")
    b_enc = nc.dram_tensor("b_enc", [DB, 128, 1], F32, kind="ExternalInput")
    w_i = nc.dram_tensor("w_i", [D, C], BF16, kind="ExternalInput")
    b_i = nc.dram_tensor("b_i", [1, C], F32, kind="ExternalInput")
    w_q1 = nc.dram_tensor("w_q1", [D, QD], BF16, kind="ExternalInput")
    b_q1 = nc.dram_tensor("b_q1", [QD, 1], F32, kind="ExternalInput")
    w_q2 = nc.dram_tensor("w_q2", [QD, QD], BF16, kind="ExternalInput")
    b_q2 = nc.dram_tensor("b_q2", [QD, 1], F32, kind="ExternalInput")
    identb_d = nc.dram_tensor("identb", [128, 128], BF16,
                              kind="ExternalInput")
    identf_d = nc.dram_tensor("identf", [128, 128], F32,
                              kind="ExternalInput")
    out_d = nc.dram_tensor("out", [C, D], F32, kind="ExternalOutput")

    groups = [[i, i + 1] for i in range(0, n_cores, 2)]

    with tile.TileContext(nc) as tc, ExitStack() as ctx:
        persist = ctx.enter_context(tc.tile_pool(name="persist", bufs=1))
        dram = ctx.enter_context(tc.tile_pool(name="dram", bufs=1,
                                              space="DRAM"))

        # warm the collective channel while phase A runs
        warm_in = dram.tile([1, 2], F32)
        nc.sync.dma_start(warm_in[:], identf_d[0:1, 0:2])
        warm_out = dram.tile([2, 2], F32)
        nc.gpsimd.collective_compute(
            "AllGather", ALU.bypass, replica_groups=groups,
            ins=[warm_in[:].opt()], outs=[warm_out[:].opt()])

        # ---- weights / constants into SBUF ----
        w_enc_sb = []
        for ib in range(IB):
            t = persist.tile([128, D], BF16, name=f"wenc{ib}")
            nc.gpsimd.dma_start(t[:], w_enc[ib * 128:(ib + 1) * 128, :])
            w_enc_sb.append(t)
        w_i_sb = []
        for db in range(DB):
            t = persist.tile([128, C], BF16, name=f"wi{db}")
            nc.gpsimd.dma_start(t[:], w_i[db * 128:(db + 1) * 128, :])
            w_i_sb.append(t)
        w_q1_sb = []
        for db in range(DB):
            t = persist.tile([128, QD], BF16, name=f"wq1{db}")
            nc.gpsimd.dma_start(t[:], w_q1[db * 128:(db + 1) * 128, :])
            w_q1_sb.append(t)
        w_q2_sb = persist.tile([128, QD], BF16)
        nc.gpsimd.dma_start(w_q2_sb[:], w_q2[:])

        identb = persist.tile([128, 128], BF16)
        nc.sync.dma_start(identb[:], identb_d[:])
        identf = persist.tile([128, 128], F32)
        nc.sync.dma_start(identf[:], identf_d[:])
        b_enc_sb = []
        for db in range(DB):
            t = persist.tile([128, 1], F32, name=f"benc{db}")
            nc.sync.dma_start(t[:], b_enc[db])
            b_enc_sb.append(t)
        b_q1_sb = persist.tile([QD, 1], F32)
        nc.sync.dma_start(b_q1_sb[:], b_q1[:])
        b_q2_sb = persist.tile([QD, 1], F32)
        nc.sync.dma_start(b_q2_sb[:], b_q2[:])
        b_i_row = persist.tile([1, C], F32)
        nc.sync.dma_start(b_i_row[:], b_i[:])
        ones_t = persist.tile([1, 128], F32)
        nc.gpsimd.memset(ones_t[:], 1.0)

        # ---- persistent activations ----
        ht_all = persist.tile([128, NCH, DB, CHUNK], BF16)   # h^T
        h_nat = persist.tile([128, NB, D], BF16)             # h natural
        qt = persist.tile([128, NCH, CHUNK], BF16)           # Q^T
        cls_nat = persist.tile([128, NB, C], F32)
        oh = persist.tile([128, NB, C], BF16)
        e_nat = persist.tile([128, NB, C], BF16)

        # ================= phase A: encoder + classes + h transposes ====
        with (
            tc.tile_pool(name="xload", bufs=2) as xload,
            tc.tile_pool(name="hp", bufs=2, space="PSUM") as hp,
            tc.tile_pool(name="tp", bufs=1, space="PSUM") as tp,
            tc.tile_pool(name="cp", bufs=2, space="PSUM") as cp,
        ):
            for cb in range(NCH):
                n0 = cb * CHUNK
                xt_c = xload.tile([128, IB, CHUNK], BF16, tag="x", name="x")
                src = xt_d[:, n0:n0 + CHUNK].rearrange("(ib p) n -> p ib n",
                                                       p=128)
                nc.sync.dma_start(xt_c[:], src)

                # h^T = relu(W_enc^T @ xT) per d-block
                for db in range(DB):
                    ph = hp.tile([128, CHUNK], F32, tag="h", name="h")
                    for ib in range(IB):
                        nc.tensor.matmul(
                            ph[:],
                            w_enc_sb[ib][:, db * 128:(db + 1) * 128],
                            xt_c[:, ib, :],
                            start=(ib == 0), stop=(ib == IB - 1))
                    nc.scalar.activation(ht_all[:, cb, db, :], ph[:],
                                         AF.Relu, bias=b_enc_sb[db][:])

                # h natural layout: 4 transposes per n-block into one tile
                for b in range(BPC):
                    nb = cb * BPC + b
                    pt = tp.tile([128, D], BF16, tag=f"t{b}", name="t")
                    for db in range(DB):
                        nc.tensor.transpose(
                            pt[:, db * 128:(db + 1) * 128],
                            ht_all[:, cb, db, b * 128:(b + 1) * 128],
                            identb[:])
                    nc.vector.tensor_copy(h_nat[:, nb, :], pt[:])

                # classes in natural layout: lhsT = h^T block, rhs = W_i
                pc = cp.tile([128, BPC, C], F32, tag="c", name="c")
                for b in range(BPC):
                    for db in range(DB):
                        nc.tensor.matmul(
                            pc[:, b, :],
                            ht_all[:, cb, db, b * 128:(b + 1) * 128],
                            w_i_sb[db][:],
                            start=(db == 0), stop=(db == DB - 1))
                nc.vector.tensor_copy(
                    cls_nat[:, cb * BPC:(cb + 1) * BPC, :], pc[:])

        # ============ phase A tail: argmax, m_feats, q_cand, gather ====
        with tc.tile_pool(name="pa", bufs=1, space="PSUM") as pa:
            # per-class max across all instances
            rmax = persist.tile([128, C], F32)
            nc.vector.reduce_max(rmax[:],
                                 cls_nat[:].rearrange("p nb c -> p c nb"),
                                 axis=mybir.AxisListType.X)
            pmv = pa.tile([C, 128], F32, name="pmv")
            nc.tensor.transpose(pmv[:], rmax[:], identf[:])
            smax = persist.tile([C, 128], F32)
            nc.vector.tensor_copy(smax[:], pmv[:])
            mval_c = persist.tile([C, 1], F32)
            nc.vector.reduce_max(mval_c[:], smax[:],
                                 axis=mybir.AxisListType.X)
            pmr = pa.tile([1, C], F32, name="pmr")
            nc.tensor.transpose(pmr[:], mval_c[:], identf[0:2, 0:2])
            mval_f = persist.tile([1, C], F32)
            nc.vector.tensor_copy(mval_f[:], pmr[:])
            pmb = pa.tile([128, C], F32, name="pmb")
            nc.tensor.matmul(pmb[:], ones_t[:], mval_f[:],
                             start=True, stop=True)
            mb = persist.tile([128, C], F32)
            nc.vector.tensor_copy(mb[:], pmb[:])

            for nb in range(NB):
                nc.vector.tensor_tensor(oh[:, nb, :], cls_nat[:, nb, :],
                                        mb[:], ALU.is_equal)

            # m = onehot^T @ h  (critical instance features)
            pmf = pa.tile([C, D], F32, name="pmf")
            for nb in range(NB):
                nc.tensor.matmul(pmf[:], oh[:, nb, :], h_nat[:, nb, :],
                                 start=(nb == 0), stop=(nb == NB - 1))
            m_sb = persist.tile([C, D], F32)
            nc.vector.tensor_copy(m_sb[:], pmf[:])
            mT = persist.tile([128, DB, C], BF16)
            for db in range(DB):
                ptm = pa.tile([128, C], F32, tag="ptm", name="ptm", bufs=2)
                nc.tensor.transpose(ptm[:],
                                    m_sb[:, db * 128:(db + 1) * 128],
                                    identf[0:2, 0:2])
                nc.vector.tensor_copy(mT[:, db, :], ptm[:])
            # q_cand = q_fn(m)
            pzm = pa.tile([128, C], F32, name="pzm")
            for db in range(DB):
                nc.tensor.matmul(pzm[:], w_q1_sb[db][:], mT[:, db, :],
                                 start=(db == 0), stop=(db == DB - 1))
            zm = persist.tile([128, C], BF16)
            nc.scalar.activation(zm[:], pzm[:], AF.Relu, bias=b_q1_sb[:])
            pqc = pa.tile([128, C], F32, name="pqc")
            nc.tensor.matmul(pqc[:], w_q2_sb[:], zm[:], start=True,
                             stop=True)
            qcand = persist.tile([128, C], F32)
            nc.scalar.activation(qcand[:], pqc[:], AF.Tanh, bias=b_q2_sb[:])

            pay1 = dram.tile([1 + 128, C], F32)
            nc.sync.dma_start(pay1[0:1, :], mval_f[:])
            nc.sync.dma_start(pay1[1:129, :], qcand[:])
            gath1 = dram.tile([2 * 129, C], F32)
            nc.gpsimd.collective_compute(
                "AllGather", ALU.bypass, replica_groups=groups,
                ins=[pay1[:].opt()], outs=[gath1[:].opt()])

        # ====== Q-pass: q_fn(h) for all chunks (hides the AllGather) ===
        with (
            tc.tile_pool(name="zp", bufs=2, space="PSUM") as zp,
            tc.tile_pool(name="qp", bufs=2, space="PSUM") as qp,
            tc.tile_pool(name="zs", bufs=2) as zs,
            tc.tile_pool(name="wp", bufs=2, space="PSUM") as wp,
        ):
            for cb in range(NCH):
                pz = zp.tile([128, CHUNK], F32, tag="z", name="z")
                for db in range(DB):
                    nc.tensor.matmul(pz[:], w_q1_sb[db][:],
                                     ht_all[:, cb, db, :],
                                     start=(db == 0), stop=(db == DB - 1))
                zt = zs.tile([128, CHUNK], BF16, tag="zt", name="zt")
                nc.vector.tensor_scalar(zt[:], pz[:], b_q1_sb[:], 0.0,
                                        ALU.add, ALU.max)
                pq = qp.tile([128, CHUNK], F32, tag="q", name="q")
                nc.tensor.matmul(pq[:], w_q2_sb[:], zt[:], start=True,
                                 stop=True)
                nc.scalar.activation(qt[:, cb, :], pq[:], AF.Tanh,
                                     bias=b_q2_sb[:])

            # keep the PE clock-gate warm while waiting on the collective
            for k in range(N_WARM):
                pw = wp.tile([128, 128], BF16, tag="w", name="w")
                nc.tensor.transpose(pw[:], identb[:], identb[:])

        # ================= phase B: winner, scores, bag output =========
        with (
            tc.tile_pool(name="ep", bufs=2, space="PSUM") as ep,
            tc.tile_pool(name="pb", bufs=1, space="PSUM") as pb,
        ):
            mv2 = persist.tile([1, 2 * C], F32)
            nc.sync.dma_start(mv2[:, 0:C], gath1[0:1, :])
            nc.sync.dma_start(mv2[:, C:2 * C], gath1[129:130, :])
            qA = persist.tile([128, C], F32)
            nc.sync.dma_start(qA[:], gath1[1:129, :])
            qB = persist.tile([128, C], F32)
            nc.sync.dma_start(qB[:], gath1[130:258, :])

            pmb2 = pb.tile([128, 2 * C], F32, name="pmb2")
            nc.tensor.matmul(pmb2[:], ones_t[:], mv2[:], start=True,
                             stop=True)
            mvb = persist.tile([128, 2 * C], F32)
            nc.vector.tensor_copy(mvb[:], pmb2[:])
            wA = persist.tile([128, C], F32)
            nc.vector.tensor_tensor(wA[:], mvb[:, 0:C], mvb[:, C:2 * C],
                                    ALU.is_ge)
            tdiff = persist.tile([128, C], F32)
            nc.vector.tensor_tensor(tdiff[:], qA[:], qB[:], ALU.subtract)
            tsel = persist.tile([128, C], F32)
            nc.vector.tensor_tensor(tsel[:], tdiff[:], wA[:], ALU.mult)
            q_win = persist.tile([128, C], BF16)
            nc.vector.tensor_tensor(q_win[:], tsel[:], qB[:], ALU.add)

            # e = exp(Q @ q_win / sqrt(qd)) in natural layout
            for cb in range(NCH):
                pe_ = ep.tile([128, BPC, C], F32, tag="e", name="e")
                for b in range(BPC):
                    nc.tensor.matmul(
                        pe_[:, b, :],
                        qt[:, cb, b * 128:(b + 1) * 128],
                        q_win[:], start=True, stop=True)
                nc.scalar.activation(
                    e_nat[:, cb * BPC:(cb + 1) * BPC, :], pe_[:],
                    AF.Exp, scale=inv_sqrt_q)

            # denominator: cross-instance then cross-partition sum
            denp = persist.tile([128, C], F32)
            nc.vector.reduce_sum(denp[:],
                                 e_nat[:].rearrange("p nb c -> p c nb"),
                                 axis=mybir.AxisListType.X)
            pdt = pb.tile([C, 128], F32, name="pdt")
            nc.tensor.transpose(pdt[:], denp[:], identf[:])
            sden = persist.tile([C, 128], F32)
            nc.vector.tensor_copy(sden[:], pdt[:])
            den = persist.tile([C, 1], F32)
            nc.vector.reduce_sum(den[:], sden[:], axis=mybir.AxisListType.X)

            # numerator: e^T @ h
            pnum = pb.tile([C, D], F32, name="pnum")
            for nb in range(NB):
                nc.tensor.matmul(pnum[:], e_nat[:, nb, :], h_nat[:, nb, :],
                                 start=(nb == 0), stop=(nb == NB - 1))
            num = persist.tile([C, D], F32)
            nc.vector.tensor_copy(num[:], pnum[:])

            pay2 = dram.tile([C, D + 1], F32)
            nc.sync.dma_start(pay2[:, 0:D], num[:])
            nc.sync.dma_start(pay2[:, D:D + 1], den[:])
            red2 = dram.tile([C, D + 1], F32)
            nc.gpsimd.collective_compute(
                "AllReduce", ALU.add, replica_groups=groups,
                ins=[pay2[:].opt()], outs=[red2[:].opt()])
            num_s = persist.tile([C, D], F32)
            nc.sync.dma_start(num_s[:], red2[:, 0:D])
            den_s = persist.tile([C, 1], F32)
            nc.sync.dma_start(den_s[:], red2[:, D:D + 1])

            recip = persist.tile([C, 1], F32)
            nc.vector.reciprocal(recip[:], den_s[:])
            out_sb = persist.tile([C, D], F32)
            nc.vector.tensor_scalar_mul(out_sb[:], num_s[:], recip[:])
            nc.sync.dma_start(out_d[:], out_sb[:])

    nc.compile()
    return nc


def _make_in_maps(inputs, n_cores=N_CORES, N_loc=N_LOC):
    x = np.asarray(inputs["x"], dtype=np.float32)
    B = x.shape[0]
    D = int(np.asarray(inputs["W_enc"]).shape[1])
    DB = D // 128

    def bf(a):
        return np.ascontiguousarray(np.asarray(a, np.float32).astype(bfdt))

    shared = {
        "w_enc": bf(inputs["W_enc"]),
        "b_enc": np.ascontiguousarray(
            np.asarray(inputs["b_enc"], np.float32).reshape(DB, 128, 1)),
        "w_i": bf(inputs["W_i"]),
        "b_i": np.ascontiguousarray(
            np.asarray(inputs["b_i"], np.float32).reshape(1, -1)),
        "w_q1": bf(inputs["W_q1"]),
        "b_q1": np.ascontiguousarray(
            np.asarray(inputs["b_q1"], np.float32).reshape(-1, 1)),
        "w_q2": bf(inputs["W_q2"]),
        "b_q2": np.ascontiguousarray(
            np.asarray(inputs["b_q2"], np.float32).reshape(-1, 1)),
        "identb": np.eye(128, dtype=np.float32).astype(bfdt),
        "identf": np.eye(128, dtype=np.float32),
    }
    xb = x.astype(bfdt)
    in_maps = []
    for core in range(n_cores):
        bag = core // 2
        half = core % 2
        xts = np.ascontiguousarray(
            xb[bag % B, half * N_loc:(half + 1) * N_loc, :].T)
        in_maps.append({"xt": xts, **shared})
    return in_maps


def kernel(**inputs) -> np.ndarray:
    from concourse.bass_utils import run_bass_kernel_spmd

    if "nc" not in _cache:
        _cache["nc"] = _build_kernel()
    nc = _cache["nc"]
    in_maps = _make_in_maps(inputs)
    res = run_bass_kernel_spmd(nc, in_maps, core_ids=list(range(N_CORES)))
    out = np.stack([res.results[2 * b]["out"] for b in range(B_BAGS)])
    return out.astype(np.float32)
